# revision 5
# baseline (speedup 1.0000x reference)
"""Multi-head attention (B=2, S=2048, D=768, H=12) on 8 Trainium2 cores.

Sharding: core c handles batch b=c//4 and heads 3*(c%4)..3*(c%4)+3.
QKV weights column-sharded, Wo row-sharded (Megatron); host sums the 4
partial outputs per batch and adds bo.

v2: whole data path in bf16 (halves HBM traffic, avoids the fp32r
<256-col matmul penalty on the V projection) and a software-pipelined
schedule: the exp activations (ACT engine, ~100us total, the kernel's
hard floor) start ~10us in and stay saturated while the PE interleaves
remaining K/V/Q projections and the previous block's output projection
into the attention loop as filler work. AV matmuls lag one group behind
the scores so the PE never stalls on exp.

Per-core layout (as v1): KT/QT[192,2048] d-major with head2 rows
duplicated to partitions 64:128 for PE row-group pairing, V[2048,192]
k-major with interleaved ones columns for the softmax denominator,
S^T[k,q] scores so softmax needs no transposes.
"""

import sys

sys.path.insert(0, "/opt/trn_rl_repo")

from contextlib import ExitStack

import ml_dtypes
import numpy as np

import concourse.bacc as bacc
import concourse.bass as bass
import concourse.tile as tile
from concourse import mybir
from concourse.bass_utils import run_bass_kernel_spmd

F32 = mybir.dt.float32
BF16 = mybir.dt.bfloat16

S = 2048  # sequence length
D = 768  # model dim
HP = 3  # heads per core
DK = 64  # head dim
DO = HP * DK  # 192 out-cols per core
KT = D // 128  # 6 contraction tiles for projections
NB = S // 512  # 4 sequence blocks of 512
NKT = S // 128  # 16 kpos tiles
G = NKT // 2  # 8 groups of 2 kpos tiles
VW = HP * 65 + 1  # 196: [1|V0|1|V1|1|V2|1] ones interleaved


def emit_kernel(nc, tc, t, reps=1):
    ctx = ExitStack()
    sb = ctx.enter_context(tc.tile_pool(name="sb", bufs=1))
    xp = ctx.enter_context(tc.tile_pool(name="xp", bufs=1))
    pt_pool = ctx.enter_context(tc.tile_pool(name="ptp", bufs=2))
    work = ctx.enter_context(tc.tile_pool(name="work", bufs=2))
    ps = ctx.enter_context(tc.tile_pool(name="ps", bufs=1, space=bass.MemorySpace.PSUM))

    # ---- persistent SBUF tensors ----
    wq_sb = sb.tile([128, KT, DO], BF16)
    wk_sb = sb.tile([128, KT, DO], BF16)
    wv_sb = sb.tile([128, KT, DO], BF16)
    wo1_sb = sb.tile([128, D], BF16)  # Wo rows 0:128
    wo2_sb = sb.tile([64, D], BF16)  # Wo rows 128:192
    bq_sb = sb.tile([128, 2], F32)
    bk_sb = sb.tile([128, 2], F32)
    bv_bc = sb.tile([128, DO], F32)  # bv broadcast to 128 partitions
    qt_a = sb.tile([128, S], BF16)  # Q^T rows 0:128 (heads 0,1)
    qt_b = sb.tile([128, S], BF16)  # Q^T rows 128:192 (head 2; dup at 64:128)
    kt_a = sb.tile([128, S], BF16)
    kt_b = sb.tile([128, S], BF16)
    v_sb = sb.tile([128, NKT, VW], BF16)  # k-major V with ones cols
    out_a = sb.tile([128, S], BF16)  # attention out^T rows 0:128
    out_b = sb.tile([64, S], BF16)  # rows 128:192

    # ---- weight / bias loads (once) ----
    nc.sync.dma_start(wk_sb[:], t["wk"].ap().rearrange("(k p) o -> p k o", p=128))
    nc.sync.dma_start(wq_sb[:], t["wq"].ap().rearrange("(k p) o -> p k o", p=128))
    nc.sync.dma_start(wv_sb[:], t["wv"].ap().rearrange("(k p) o -> p k o", p=128))
    nc.sync.dma_start(wo1_sb[:], t["wo"].ap()[0:128, :])
    nc.sync.dma_start(wo2_sb[:], t["wo"].ap()[128:DO, :])
    nc.sync.dma_start(bq_sb[:, 0:1], t["bq"].ap()[0:128, :])
    nc.sync.dma_start(bq_sb[0:64, 1:2], t["bq"].ap()[128:DO, :])
    nc.sync.dma_start(bk_sb[:, 0:1], t["bk"].ap()[0:128, :])
    nc.sync.dma_start(bk_sb[0:64, 1:2], t["bk"].ap()[128:DO, :])
    nc.sync.dma_start(bv_bc[:], t["bv"].ap().unsqueeze(1).to_broadcast([1, 128, DO]))
    for oc in (0, 65, 130, 195):
        nc.vector.memset(v_sb[:, :, oc : oc + 1], 1.0)

    # ---- per-tensor x tiles (all three live concurrently now) ----
    xk_t = [xp.tile([128, S], BF16, name=f"xk{k}", tag=f"xk{k}") for k in range(KT)]
    xq_t = [xp.tile([128, S], BF16, name=f"xq{k}", tag=f"xq{k}") for k in range(KT)]
    xv_t = [xp.tile([128, S], BF16, name=f"xv{k}", tag=f"xv{k}") for k in range(KT)]

    def load_chunk(tiles, dram, c):
        c0 = c * 1024
        for k in range(KT):
            nc.gpsimd.dma_start(
                tiles[k][:, c0 : c0 + 1024],
                dram.ap()[k * 128 : k * 128 + 128, c0 : c0 + 1024],
            )

    def proj_qk_a(xts, w_sb, b_sb, dst_a, nb):
        """128-row half of a Q/K projection chunk (heads 0,1 dims)."""
        nb0 = nb * 512
        pq1 = ps.tile([128, 2, 512], F32, tag="A", bufs=2, name="pq1")
        for k in range(KT):
            nc.tensor.matmul(
                pq1[:, 0, :], w_sb[:, k, 0:128], xts[k][:, nb0 : nb0 + 512],
                start=(k == 0), stop=(k == KT - 1),
            )
        nc.vector.tensor_scalar_add(
            dst_a[:, nb0 : nb0 + 512], pq1[:, 0, :], b_sb[:, 0:1]
        )

    def proj_qk_b(xts, w_sb, b_sb, dst_b, nb):
        """64-row half (head 2) + duplication into partitions 64:128."""
        nb0 = nb * 512
        pq2 = ps.tile([128, 512], F32, tag="B", bufs=1, name="pq2")
        for k in range(KT):
            nc.tensor.matmul(
                pq2[0:64, :], w_sb[:, k, 128:DO], xts[k][:, nb0 : nb0 + 512],
                start=(k == 0), stop=(k == KT - 1),
            )
        nc.vector.tensor_scalar_add(
            dst_b[0:64, nb0 : nb0 + 512], pq2[0:64, :], b_sb[0:64, 1:2]
        )
        nc.sync.dma_start(
            dst_b[64:128, nb0 : nb0 + 512], dst_b[0:64, nb0 : nb0 + 512]
        )

    def proj_qk_nb(xts, w_sb, b_sb, dst_a, dst_b, nb):
        proj_qk_a(xts, w_sb, b_sb, dst_a, nb)
        proj_qk_b(xts, w_sb, b_sb, dst_b, nb)

    def proj_v_st(st):
        pv = ps.tile([128, 512], F32, tag="B", bufs=1, name="pv")
        for k in range(KT):
            nc.tensor.matmul(
                pv[:, 0:DO], xv_t[k][:, st * 128 : st * 128 + 128], wv_sb[:, k, :],
                start=(k == 0), stop=(k == KT - 1),
            )
        dst = v_sb[:, st, 1:196].rearrange("p (h c) -> p h c", h=HP)[:, :, 0:64]
        nc.vector.tensor_add(dst, pv[:, 0:DO].rearrange("p (h c) -> p h c", h=HP),
                             bv_bc[:].rearrange("p (h c) -> p h c", h=HP))

    def vslice(kt_i, h):
        return v_sb[:, kt_i, 1 + h * 65 : 1 + h * 65 + 65]

    def normalize(po, h, q0):
        """row 64 of po = denominator; write normalized out^T rows."""
        dtmp = work.tile([65, 512], F32, name="dtmp", tag="dtmp")
        nc.vector.reciprocal(dtmp[64:65, :], po[64:65, :])
        dbc = work.tile([64, 512], F32, name="dbc", tag="dbc")
        nc.sync.dma_start(dbc[:], dtmp[64:65, :].unsqueeze(1).to_broadcast([1, 64, 512]))
        if h == 0:
            nc.vector.tensor_mul(out_a[0:64, q0 : q0 + 512], po[0:64, :], dbc[:])
        elif h == 2:
            nc.vector.tensor_mul(out_b[0:64, q0 : q0 + 512], po[0:64, :], dbc[:])
        else:
            nsb = work.tile([64, 512], BF16, name="nsb", tag="nsb")
            nc.vector.tensor_mul(nsb[:], po[0:64, :], dbc[:])
            nc.sync.dma_start(out_a[64:128, q0 : q0 + 512], nsb[:])

    def outproj_ss(qb, ss):
        s0 = qb * 512 + ss * 128
        ysb = work.tile([128, D], BF16, name="ysb", tag="ysb")
        for nc0 in (0, 384):
            py = ps.tile([128, 384], F32, tag="C", bufs=1, name="py")
            nc.tensor.matmul(
                py[:], out_a[:, s0 : s0 + 128],
                wo1_sb[:, nc0 : nc0 + 384], start=True, stop=False,
            )
            nc.tensor.matmul(
                py[:], out_b[:, s0 : s0 + 128],
                wo2_sb[:, nc0 : nc0 + 384], start=False, stop=True,
            )
            nc.vector.tensor_copy(ysb[:, nc0 : nc0 + 384], py[:])
        nc.sync.dma_start(t["y"].ap()[s0 : s0 + 128, :], ysb[:])

    def emit_av(g, pt0, pt1, po0, po1):
        for kl in range(2):
            kt_i = g * 2 + kl
            nc.tensor.matmul(
                po0[:], vslice(kt_i, 0), pt0[:, kl, :],
                start=(kt_i == 0), stop=(kt_i == NKT - 1),
                skip_group_check=True,
            )
            nc.tensor.matmul(
                po1[:], vslice(kt_i, 1), pt1[:, kl, :],
                start=(kt_i == 0), stop=(kt_i == NKT - 1),
                skip_group_check=True,
            )

    def emit_av2(h, pt2, po2):
        for kl in range(2):
            kt_i = h * 2 + kl
            nc.tensor.matmul(
                po2[:], vslice(kt_i, 2), pt2[:, kl, :],
                start=(kt_i == 0), stop=(kt_i == NKT - 1),
                skip_group_check=True,
            )

    def attention_qb(qb, fillers_g, fillers_h):
        q0 = qb * 512
        po0 = ps.tile([65, 512], F32, tag="B2", bufs=2, name="po0")
        po1 = ps.tile([65, 512], F32, tag="B2", bufs=2, name="po1")
        pts = [None] * G
        for g in range(G):
            pss0 = ps.tile([128, 2, 512], F32, tag="A", bufs=2, name="pss0")
            pss1 = ps.tile([128, 2, 512], F32, tag="A", bufs=2, name="pss1")
            for kl in range(2):
                kk = (g * 2 + kl) * 128
                nc.tensor.matmul(
                    pss0[:, kl, :], kt_a[0:64, kk : kk + 128],
                    qt_a[0:64, q0 : q0 + 512], start=True, stop=True,
                )
                nc.tensor.matmul(
                    pss1[:, kl, :], kt_a[64:128, kk : kk + 128],
                    qt_a[64:128, q0 : q0 + 512], start=True, stop=True,
                )
            pt0 = pt_pool.tile([128, 2, 512], BF16, name="pt0", tag="pt0")
            nc.scalar.activation(
                pt0[:], pss0[:], mybir.ActivationFunctionType.Exp, scale=0.125
            )
            pt1 = pt_pool.tile([128, 2, 512], BF16, name="pt1", tag="pt1")
            nc.scalar.activation(
                pt1[:], pss1[:], mybir.ActivationFunctionType.Exp, scale=0.125
            )
            pts[g] = (pt0, pt1)
            for f in fillers_g.get(g, ()):
                f()
            if g > 0:
                emit_av(g - 1, *pts[g - 1], po0, po1)
        emit_av(G - 1, *pts[G - 1], po0, po1)
        normalize(po0, 0, q0)
        normalize(po1, 1, q0)
        # head 2: pair even/odd kt via the duplicated rows 64:128
        po2 = ps.tile([65, 512], F32, tag="B2", bufs=2, name="po2")
        pt2s = [None] * G
        for h in range(G):
            pss2 = ps.tile([128, 2, 512], F32, tag="A", bufs=2, name="pss2")
            kk = h * 256
            nc.tensor.matmul(
                pss2[:, 0, :], kt_b[0:64, kk : kk + 128],
                qt_b[0:64, q0 : q0 + 512], start=True, stop=True,
            )
            nc.tensor.matmul(
                pss2[:, 1, :], kt_b[64:128, kk + 128 : kk + 256],
                qt_b[64:128, q0 : q0 + 512], start=True, stop=True,
            )
            pt2 = pt_pool.tile([128, 2, 512], BF16, name="pt2", tag="pt2")
            nc.scalar.activation(
                pt2[:], pss2[:], mybir.ActivationFunctionType.Exp, scale=0.125
            )
            pt2s[h] = pt2
            for f in fillers_h.get(h, ()):
                f()
            if h > 0:
                emit_av2(h - 1, pt2s[h - 1], po2)
        emit_av2(G - 1, pt2s[G - 1], po2)
        normalize(po2, 2, q0)

    def K(nb):
        return lambda: proj_qk_nb(xk_t, wk_sb, bk_sb, kt_a, kt_b, nb)

    def Qa(nb):
        return lambda: proj_qk_a(xq_t, wq_sb, bq_sb, qt_a, nb)

    def Qb(nb):
        return lambda: proj_qk_b(xq_t, wq_sb, bq_sb, qt_b, nb)

    def V(st):
        return lambda: proj_v_st(st)

    def O(qb, s0, s1):
        return lambda: [outproj_ss(qb, ss) for ss in (s0, s1)]

    for _ in range(reps):
        # DMA issue order: critical-path chunks first (c0 = kpos 0:1024)
        load_chunk(xk_t, t["xk"], 0)
        load_chunk(xq_t, t["xq"], 0)
        load_chunk(xv_t, t["xv"], 0)
        load_chunk(xk_t, t["xk"], 1)
        load_chunk(xq_t, t["xq"], 1)
        load_chunk(xv_t, t["xv"], 1)

        # warmup: enough K/Q/V for the first attention groups
        proj_qk_nb(xk_t, wk_sb, bk_sb, kt_a, kt_b, 0)
        proj_qk_a(xq_t, wq_sb, bq_sb, qt_a, 0)
        proj_qk_b(xq_t, wq_sb, bq_sb, qt_b, 0)
        for st in range(4):
            proj_v_st(st)

        attention_qb(
            0,
            fillers_g={
                0: [K(1)],
                1: [V(4), V(5)],
                2: [K(2)],
                3: [V(6), V(7)],
                4: [K(3)],
                5: [V(8), V(9)],
                6: [V(10), V(11)],
                7: [V(12), V(13), V(14), V(15)],
            },
            fillers_h={0: [Qa(1)], 1: [Qb(1)]},
        )
        attention_qb(
            1,
            fillers_g={0: [Qa(2)], 1: [Qb(2)], 2: [O(0, 0, 1)], 3: [O(0, 2, 3)]},
            fillers_h={},
        )
        attention_qb(
            2,
            fillers_g={0: [Qa(3)], 1: [Qb(3)], 2: [O(1, 0, 1)], 3: [O(1, 2, 3)]},
            fillers_h={},
        )
        attention_qb(
            3,
            fillers_g={2: [O(2, 0, 1)], 3: [O(2, 2, 3)]},
            fillers_h={},
        )
        for ss in range(4):
            outproj_ss(3, ss)

    ctx.close()


_NC_CACHE = {}


def build_nc(reps=1):
    if reps in _NC_CACHE:
        return _NC_CACHE[reps]
    nc = bacc.Bacc("TRN2", target_bir_lowering=False, debug=False, num_devices=8)
    t = {}
    for name in ("xq", "xk", "xv"):
        t[name] = nc.dram_tensor(name, [D, S], BF16, kind="ExternalInput")
    for name in ("wq", "wk", "wv"):
        t[name] = nc.dram_tensor(name, [D, DO], BF16, kind="ExternalInput")
    t["wo"] = nc.dram_tensor("wo", [DO, D], BF16, kind="ExternalInput")
    for name in ("bq", "bk"):
        t[name] = nc.dram_tensor(name, [DO, 1], F32, kind="ExternalInput")
    t["bv"] = nc.dram_tensor("bv", [1, DO], F32, kind="ExternalInput")
    t["y"] = nc.dram_tensor("y", [S, D], BF16, kind="ExternalOutput")

    with tile.TileContext(nc) as tc:
        emit_kernel(nc, tc, t, reps=reps)
    nc.compile()
    _NC_CACHE[reps] = nc
    return nc


def make_in_maps(q, k, v, Wq, bq, Wk, bk, Wv, bv, Wo, bo):
    bf = ml_dtypes.bfloat16
    in_maps = []
    for c in range(8):
        b = c // 4
        hs = (c % 4) * DO
        in_maps.append(
            {
                "xq": np.ascontiguousarray(q[b].T).astype(bf),
                "xk": np.ascontiguousarray(k[b].T).astype(bf),
                "xv": np.ascontiguousarray(v[b].T).astype(bf),
                "wq": np.ascontiguousarray(Wq[:, hs : hs + DO]).astype(bf),
                "wk": np.ascontiguousarray(Wk[:, hs : hs + DO]).astype(bf),
                "wv": np.ascontiguousarray(Wv[:, hs : hs + DO]).astype(bf),
                "wo": np.ascontiguousarray(Wo[hs : hs + DO, :]).astype(bf),
                "bq": np.ascontiguousarray(bq[hs : hs + DO, None]).astype(np.float32),
                "bk": np.ascontiguousarray(bk[hs : hs + DO, None]).astype(np.float32),
                "bv": np.ascontiguousarray(bv[None, hs : hs + DO]).astype(np.float32),
            }
        )
    return in_maps


def kernel(q, k, v, Wq, bq, Wk, bk, Wv, bv, Wo, bo, _reps=1):
    q = np.asarray(q, dtype=np.float32)
    k = np.asarray(k, dtype=np.float32)
    v = np.asarray(v, dtype=np.float32)
    nc = build_nc(reps=_reps)
    in_maps = make_in_maps(q, k, v, np.asarray(Wq), np.asarray(bq), np.asarray(Wk),
                           np.asarray(bk), np.asarray(Wv), np.asarray(bv),
                           np.asarray(Wo), np.asarray(bo))
    res = run_bass_kernel_spmd(nc, in_maps, list(range(8)))
    B = q.shape[0]
    y = np.zeros((B, S, D), dtype=np.float32)
    for c in range(8):
        y[c // 4] += np.asarray(res.results[c]["y"], dtype=np.float32)
    y += np.asarray(bo, dtype=np.float32)[None, None, :]
    return y


# revision 11
# speedup vs baseline: 1.2291x; 1.2291x over previous
"""Multi-head attention (B=2, S=2048, D=768, H=12) on 8 Trainium2 cores.

Sharding: core c handles batch b=c//4 and heads 3*(c%4)..3*(c%4)+3.
QKV weights column-sharded, Wo row-sharded (Megatron); host sums the 4
partial outputs per batch and adds bo.

v3: measured on HW that 64-contraction matmuls cost ~2x per streamed
column vs full 128-contraction ones (563ns/pair vs 143.6ns single), so
every matmul here is built as a full 128-contraction:
  - Q^T stored zero-padded per head (qt_z0=[qh0;0], qt_z1=[0;qh1],
    qt_z2=[qh2;0]); K^T packed (kt_a=[kh0;kh1], kt_b=[kh2;0]). Scores
    for head h are then single full matmuls sharing the kt weights.
  - out_b / wo2 (head-2 rows of the output projection) zero-padded to
    128 partitions.
Everything bf16 (halves DMA, no fp32r short-stream penalty), exp on ACT
(measured 582ns per [128,1024] tile -> 56us/rep floor), and the PE
stream is software-pipelined: AV lags exp by one group, projections and
the previous block's output projection fill the gaps.
"""

import sys

sys.path.insert(0, "/opt/trn_rl_repo")

from contextlib import ExitStack

import ml_dtypes
import numpy as np

import concourse.bacc as bacc
import concourse.bass as bass
import concourse.tile as tile
from concourse import mybir
from concourse.bass_utils import run_bass_kernel_spmd

F32 = mybir.dt.float32
BF16 = mybir.dt.bfloat16

S = 2048  # sequence length
D = 768  # model dim
HP = 3  # heads per core
DK = 64  # head dim
DO = HP * DK  # 192 out-cols per core
KT = D // 128  # 6 contraction tiles for projections
NB = S // 512  # 4 sequence blocks of 512
NKT = S // 128  # 16 kpos tiles
G = NKT // 2  # 8 groups of 2 kpos tiles
VW = HP * 65 + 1  # 196: [1|V0|1|V1|1|V2|1] ones interleaved


def emit_kernel(nc, tc, t, reps=1):
    ctx = ExitStack()
    sb = ctx.enter_context(tc.tile_pool(name="sb", bufs=1))
    xp = ctx.enter_context(tc.tile_pool(name="xp", bufs=1))
    pt_pool = ctx.enter_context(tc.tile_pool(name="ptp", bufs=2))
    work = ctx.enter_context(tc.tile_pool(name="work", bufs=2))
    ps = ctx.enter_context(tc.tile_pool(name="ps", bufs=1, space=bass.MemorySpace.PSUM))

    # ---- persistent SBUF tensors ----
    wq_sb = sb.tile([128, KT, DO], BF16)
    wk_sb = sb.tile([128, KT, DO], BF16)
    # rhs operands padded to 512 streamed cols: short streams measured ~3x
    # slower per column on HW (192-col matmul 424ns vs 512-col 143.6ns)
    wv_sb = sb.tile([128, KT, 512], BF16)  # wv in cols 0:192, zeros after
    wo1_sb = sb.tile([128, 2, 512], BF16)  # Wo rows 0:128, 768 cols + pad
    wo2_sb = sb.tile([128, 2, 512], BF16)  # Wo rows 128:192 at 0:64, zeros
    bq_sb = sb.tile([128, 2], F32)
    bk_sb = sb.tile([128, 2], F32)
    bv_bc = sb.tile([128, DO], F32)  # bv broadcast to 128 partitions
    qt_z0 = sb.tile([128, S], BF16)  # [Q^T h0; 0]
    qt_z1 = sb.tile([128, S], BF16)  # [0; Q^T h1]
    qt_z2 = sb.tile([128, S], BF16)  # [Q^T h2; 0]
    kt_a = sb.tile([128, S], BF16)  # [K^T h0; K^T h1]
    kt_b = sb.tile([128, S], BF16)  # [K^T h2; 0]
    v_sb = sb.tile([128, NKT, VW], BF16)  # k-major V with ones cols
    out_a = sb.tile([128, S], BF16)  # attention out^T rows 0:128 (h0,h1)
    out_b = sb.tile([128, S], BF16)  # rows 128:192 (h2) at 0:64, zeros below

    # ---- weight / bias loads + one-time zero/ones fills ----
    nc.sync.dma_start(wk_sb[:], t["wk"].ap().rearrange("(k p) o -> p k o", p=128))
    nc.sync.dma_start(wq_sb[:], t["wq"].ap().rearrange("(k p) o -> p k o", p=128))
    nc.sync.dma_start(
        wv_sb[:, :, 0:DO], t["wv"].ap().rearrange("(k p) o -> p k o", p=128)
    )
    nc.sync.dma_start(wo1_sb[:, 0, :], t["wo"].ap()[0:128, 0:512])
    nc.sync.dma_start(wo1_sb[:, 1, 0:256], t["wo"].ap()[0:128, 512:768])
    nc.sync.dma_start(wo2_sb[0:64, 0, :], t["wo"].ap()[128:DO, 0:512])
    nc.sync.dma_start(wo2_sb[0:64, 1, 0:256], t["wo"].ap()[128:DO, 512:768])
    nc.sync.dma_start(bq_sb[:, 0:1], t["bq"].ap()[0:128, :])
    nc.sync.dma_start(bq_sb[0:64, 1:2], t["bq"].ap()[128:DO, :])
    nc.sync.dma_start(bk_sb[:, 0:1], t["bk"].ap()[0:128, :])
    nc.sync.dma_start(bk_sb[0:64, 1:2], t["bk"].ap()[128:DO, :])
    nc.sync.dma_start(bv_bc[:], t["bv"].ap().unsqueeze(1).to_broadcast([1, 128, DO]))
    nc.vector.memset(wv_sb[:, :, DO:512], 0.0)
    nc.vector.memset(wo1_sb[:, 1, 256:512], 0.0)
    nc.vector.memset(wo2_sb[64:128, :, :], 0.0)
    nc.vector.memset(wo2_sb[0:64, 1, 256:512], 0.0)
    nc.vector.memset(qt_z0[64:128, :], 0.0)
    nc.vector.memset(qt_z1[0:64, :], 0.0)
    nc.vector.memset(qt_z2[64:128, :], 0.0)
    nc.vector.memset(kt_b[64:128, :], 0.0)
    nc.vector.memset(out_b[64:128, :], 0.0)
    for oc in (0, 65, 130, 195):
        nc.vector.memset(v_sb[:, :, oc : oc + 1], 1.0)

    # ---- per-tensor x tiles (all three live concurrently) ----
    xk_t = [xp.tile([128, S], BF16, name=f"xk{k}", tag=f"xk{k}") for k in range(KT)]
    xq_t = [xp.tile([128, S], BF16, name=f"xq{k}", tag=f"xq{k}") for k in range(KT)]
    xv_t = [xp.tile([128, S], BF16, name=f"xv{k}", tag=f"xv{k}") for k in range(KT)]

    def load_chunk(tiles, dram, c0, w):
        for k in range(KT):
            nc.gpsimd.dma_start(
                tiles[k][:, c0 : c0 + w],
                dram.ap()[k * 128 : k * 128 + 128, c0 : c0 + w],
            )

    def proj_k_a(nb):
        """K heads 0,1 -> kt_a (packed)."""
        nb0 = nb * 512
        pq1 = ps.tile([128, 2, 512], F32, tag="A", bufs=2, name="pq1")
        for k in range(KT):
            nc.tensor.matmul(
                pq1[:, 0, :], wk_sb[:, k, 0:128], xk_t[k][:, nb0 : nb0 + 512],
                start=(k == 0), stop=(k == KT - 1),
            )
        nc.vector.tensor_scalar_add(
            kt_a[:, nb0 : nb0 + 512], pq1[:, 0, :], bk_sb[:, 0:1]
        )

    def proj_k_b(nb):
        """K head 2 -> kt_b rows 0:64."""
        nb0 = nb * 512
        pq2 = ps.tile([128, 512], F32, tag="B", bufs=1, name="pq2")
        for k in range(KT):
            nc.tensor.matmul(
                pq2[0:64, :], wk_sb[:, k, 128:DO], xk_t[k][:, nb0 : nb0 + 512],
                start=(k == 0), stop=(k == KT - 1),
            )
        nc.vector.tensor_scalar_add(
            kt_b[0:64, nb0 : nb0 + 512], pq2[0:64, :], bk_sb[0:64, 1:2]
        )

    def proj_q_a(nb):
        """Q heads 0,1 -> zero-padded qt_z0 / qt_z1."""
        nb0 = nb * 512
        pq1 = ps.tile([128, 2, 512], F32, tag="A", bufs=2, name="pq1")
        for k in range(KT):
            nc.tensor.matmul(
                pq1[:, 0, :], wq_sb[:, k, 0:128], xq_t[k][:, nb0 : nb0 + 512],
                start=(k == 0), stop=(k == KT - 1),
            )
        nc.vector.tensor_scalar_add(
            qt_z0[0:64, nb0 : nb0 + 512], pq1[0:64, 0, :], bq_sb[0:64, 0:1]
        )
        nc.vector.tensor_scalar_add(
            qt_z1[64:128, nb0 : nb0 + 512], pq1[64:128, 0, :], bq_sb[64:128, 0:1]
        )

    def proj_q_b(nb):
        """Q head 2 -> qt_z2 rows 0:64."""
        nb0 = nb * 512
        pq2 = ps.tile([128, 512], F32, tag="B", bufs=1, name="pq2")
        for k in range(KT):
            nc.tensor.matmul(
                pq2[0:64, :], wq_sb[:, k, 128:DO], xq_t[k][:, nb0 : nb0 + 512],
                start=(k == 0), stop=(k == KT - 1),
            )
        nc.vector.tensor_scalar_add(
            qt_z2[0:64, nb0 : nb0 + 512], pq2[0:64, :], bq_sb[0:64, 1:2]
        )

    def proj_v_st(st):
        pv = ps.tile([128, 512], F32, tag="B", bufs=1, name="pv")
        for k in range(KT):
            nc.tensor.matmul(
                pv[:], xv_t[k][:, st * 128 : st * 128 + 128], wv_sb[:, k, :],
                start=(k == 0), stop=(k == KT - 1),
            )
        dst = v_sb[:, st, 1:196].rearrange("p (h c) -> p h c", h=HP)[:, :, 0:64]
        nc.vector.tensor_add(dst, pv[:, 0:DO].rearrange("p (h c) -> p h c", h=HP),
                             bv_bc[:].rearrange("p (h c) -> p h c", h=HP))

    def vslice(kt_i, h):
        return v_sb[:, kt_i, 1 + h * 65 : 1 + h * 65 + 65]

    def normalize(po, h, q0):
        """row 64 of po = denominator; write normalized out^T rows."""
        dtmp = work.tile([65, 512], F32, name="dtmp", tag="dtmp")
        nc.vector.reciprocal(dtmp[64:65, :], po[64:65, :])
        dbc = work.tile([64, 512], F32, name="dbc", tag="dbc")
        nc.sync.dma_start(dbc[:], dtmp[64:65, :].unsqueeze(1).to_broadcast([1, 64, 512]))
        if h == 0:
            nc.vector.tensor_mul(out_a[0:64, q0 : q0 + 512], po[0:64, :], dbc[:])
        elif h == 2:
            nc.vector.tensor_mul(out_b[0:64, q0 : q0 + 512], po[0:64, :], dbc[:])
        else:
            nsb = work.tile([64, 512], BF16, name="nsb", tag="nsb")
            nc.vector.tensor_mul(nsb[:], po[0:64, :], dbc[:])
            nc.sync.dma_start(out_a[64:128, q0 : q0 + 512], nsb[:])

    def outproj_ss(qb, ss):
        s0 = qb * 512 + ss * 128
        ysb = work.tile([128, D], BF16, name="ysb", tag="ysb")
        for c, w in ((0, 512), (1, 256)):
            py = ps.tile([128, 512], F32, tag="C", bufs=1, name="py")
            nc.tensor.matmul(
                py[:], out_a[:, s0 : s0 + 128],
                wo1_sb[:, c, :], start=True, stop=False,
            )
            nc.tensor.matmul(
                py[:], out_b[:, s0 : s0 + 128],
                wo2_sb[:, c, :], start=False, stop=True,
            )
            nc.vector.tensor_copy(ysb[:, c * 512 : c * 512 + w], py[:, 0:w])
        nc.sync.dma_start(t["y"].ap()[s0 : s0 + 128, :], ysb[:])

    def emit_av(g, pt0, pt1, po0, po1):
        for kl in range(2):
            kt_i = g * 2 + kl
            nc.tensor.matmul(
                po0[:], vslice(kt_i, 0), pt0[:, kl, :],
                start=(kt_i == 0), stop=(kt_i == NKT - 1),
                skip_group_check=True,
            )
            nc.tensor.matmul(
                po1[:], vslice(kt_i, 1), pt1[:, kl, :],
                start=(kt_i == 0), stop=(kt_i == NKT - 1),
                skip_group_check=True,
            )

    def emit_av2(h, pt2, po2):
        for kl in range(2):
            kt_i = h * 2 + kl
            nc.tensor.matmul(
                po2[:], vslice(kt_i, 2), pt2[:, kl, :],
                start=(kt_i == 0), stop=(kt_i == NKT - 1),
                skip_group_check=True,
            )

    def attention_qb(qb, fillers_g, fillers_h):
        q0 = qb * 512
        po0 = ps.tile([65, 512], F32, tag="B2", bufs=2, name="po0")
        po1 = ps.tile([65, 512], F32, tag="B2", bufs=2, name="po1")
        pts = [None] * G
        for g in range(G):
            pss0 = ps.tile([128, 2, 512], F32, tag="A", bufs=2, name="pss0")
            pss1 = ps.tile([128, 2, 512], F32, tag="A", bufs=2, name="pss1")
            for kl in range(2):
                kk = (g * 2 + kl) * 128
                nc.tensor.matmul(
                    pss0[:, kl, :], kt_a[:, kk : kk + 128],
                    qt_z0[:, q0 : q0 + 512], start=True, stop=True,
                )
                nc.tensor.matmul(
                    pss1[:, kl, :], kt_a[:, kk : kk + 128],
                    qt_z1[:, q0 : q0 + 512], start=True, stop=True,
                )
            pt0 = pt_pool.tile([128, 2, 512], BF16, name="pt0", tag="pt0")
            nc.scalar.activation(
                pt0[:], pss0[:], mybir.ActivationFunctionType.Exp, scale=0.125
            )
            pt1 = pt_pool.tile([128, 2, 512], BF16, name="pt1", tag="pt1")
            nc.scalar.activation(
                pt1[:], pss1[:], mybir.ActivationFunctionType.Exp, scale=0.125
            )
            pts[g] = (pt0, pt1)
            for f in fillers_g.get(g, ()):
                f()
            if g > 0:
                emit_av(g - 1, *pts[g - 1], po0, po1)
        emit_av(G - 1, *pts[G - 1], po0, po1)
        normalize(po0, 0, q0)
        normalize(po1, 1, q0)
        # head 2
        po2 = ps.tile([65, 512], F32, tag="B2", bufs=2, name="po2")
        pt2s = [None] * G
        for h in range(G):
            pss2 = ps.tile([128, 2, 512], F32, tag="A", bufs=2, name="pss2")
            for kl in range(2):
                kk = (h * 2 + kl) * 128
                nc.tensor.matmul(
                    pss2[:, kl, :], kt_b[:, kk : kk + 128],
                    qt_z2[:, q0 : q0 + 512], start=True, stop=True,
                )
            pt2 = pt_pool.tile([128, 2, 512], BF16, name="pt2", tag="pt2")
            nc.scalar.activation(
                pt2[:], pss2[:], mybir.ActivationFunctionType.Exp, scale=0.125
            )
            pt2s[h] = pt2
            for f in fillers_h.get(h, ()):
                f()
            if h > 0:
                emit_av2(h - 1, pt2s[h - 1], po2)
        emit_av2(G - 1, pt2s[G - 1], po2)
        normalize(po2, 2, q0)

    def Ka(nb):
        return lambda: proj_k_a(nb)

    def Kb(nb):
        return lambda: proj_k_b(nb)

    def Qa(nb):
        return lambda: proj_q_a(nb)

    def Qb(nb):
        return lambda: proj_q_b(nb)

    def V(st):
        return lambda: proj_v_st(st)

    def O(qb, ss):
        return lambda: outproj_ss(qb, ss)

    for _ in range(reps):
        # DMA issue order: critical-path chunks first
        load_chunk(xk_t, t["xk"], 0, 1024)
        load_chunk(xq_t, t["xq"], 0, 1024)
        load_chunk(xv_t, t["xv"], 0, 512)
        load_chunk(xv_t, t["xv"], 512, 512)
        load_chunk(xk_t, t["xk"], 1024, 1024)
        load_chunk(xq_t, t["xq"], 1024, 1024)
        load_chunk(xv_t, t["xv"], 1024, 1024)

        # warmup: just enough K/Q for the first score groups
        proj_k_a(0)
        proj_q_a(0)

        attention_qb(
            0,
            fillers_g={
                1: [Ka(1), V(0), V(1)],
                2: [V(2), V(3)],
                3: [Ka(2), V(4), V(5)],
                4: [V(6), V(7)],
                5: [Ka(3), V(8), V(9)],
                6: [Kb(0), Kb(1), V(10), V(11)],
                7: [Kb(2), Kb(3), V(12), V(13), V(14), V(15), Qb(0)],
            },
            fillers_h={0: [Qa(1)], 1: [Qb(1)]},
        )
        attention_qb(
            1,
            fillers_g={0: [Qa(2)], 1: [Qb(2)],
                       2: [O(0, 0), O(0, 1)], 3: [O(0, 2), O(0, 3)]},
            fillers_h={},
        )
        attention_qb(
            2,
            fillers_g={0: [Qa(3)], 1: [Qb(3)],
                       2: [O(1, 0), O(1, 1)], 3: [O(1, 2), O(1, 3)]},
            fillers_h={},
        )
        attention_qb(
            3,
            fillers_g={2: [O(2, 0), O(2, 1)], 3: [O(2, 2), O(2, 3)]},
            fillers_h={},
        )
        for ss in range(4):
            outproj_ss(3, ss)

    ctx.close()


_NC_CACHE = {}


def build_nc(reps=1):
    if reps in _NC_CACHE:
        return _NC_CACHE[reps]
    nc = bacc.Bacc("TRN2", target_bir_lowering=False, debug=False, num_devices=8)
    t = {}
    for name in ("xq", "xk", "xv"):
        t[name] = nc.dram_tensor(name, [D, S], BF16, kind="ExternalInput")
    for name in ("wq", "wk", "wv"):
        t[name] = nc.dram_tensor(name, [D, DO], BF16, kind="ExternalInput")
    t["wo"] = nc.dram_tensor("wo", [DO, D], BF16, kind="ExternalInput")
    for name in ("bq", "bk"):
        t[name] = nc.dram_tensor(name, [DO, 1], F32, kind="ExternalInput")
    t["bv"] = nc.dram_tensor("bv", [1, DO], F32, kind="ExternalInput")
    t["y"] = nc.dram_tensor("y", [S, D], BF16, kind="ExternalOutput")

    with tile.TileContext(nc) as tc:
        emit_kernel(nc, tc, t, reps=reps)
    nc.compile()
    _NC_CACHE[reps] = nc
    return nc


def make_in_maps(q, k, v, Wq, bq, Wk, bk, Wv, bv, Wo, bo):
    bf = ml_dtypes.bfloat16
    in_maps = []
    for c in range(8):
        b = c // 4
        hs = (c % 4) * DO
        in_maps.append(
            {
                "xq": np.ascontiguousarray(q[b].T).astype(bf),
                "xk": np.ascontiguousarray(k[b].T).astype(bf),
                "xv": np.ascontiguousarray(v[b].T).astype(bf),
                "wq": np.ascontiguousarray(Wq[:, hs : hs + DO]).astype(bf),
                "wk": np.ascontiguousarray(Wk[:, hs : hs + DO]).astype(bf),
                "wv": np.ascontiguousarray(Wv[:, hs : hs + DO]).astype(bf),
                "wo": np.ascontiguousarray(Wo[hs : hs + DO, :]).astype(bf),
                "bq": np.ascontiguousarray(bq[hs : hs + DO, None]).astype(np.float32),
                "bk": np.ascontiguousarray(bk[hs : hs + DO, None]).astype(np.float32),
                "bv": np.ascontiguousarray(bv[None, hs : hs + DO]).astype(np.float32),
            }
        )
    return in_maps


def kernel(q, k, v, Wq, bq, Wk, bk, Wv, bv, Wo, bo, _reps=1):
    q = np.asarray(q, dtype=np.float32)
    k = np.asarray(k, dtype=np.float32)
    v = np.asarray(v, dtype=np.float32)
    nc = build_nc(reps=_reps)
    in_maps = make_in_maps(q, k, v, np.asarray(Wq), np.asarray(bq), np.asarray(Wk),
                           np.asarray(bk), np.asarray(Wv), np.asarray(bv),
                           np.asarray(Wo), np.asarray(bo))
    res = run_bass_kernel_spmd(nc, in_maps, list(range(8)))
    B = q.shape[0]
    y = np.zeros((B, S, D), dtype=np.float32)
    for c in range(8):
        y[c // 4] += np.asarray(res.results[c]["y"], dtype=np.float32)
    y += np.asarray(bo, dtype=np.float32)[None, None, :]
    return y


# revision 19
# speedup vs baseline: 1.3092x; 1.0652x over previous
"""Multi-head attention (B=2, S=2048, D=768, H=12) on 8 Trainium2 cores.

Sharding: core c handles batch b=c//4 and heads 3*(c%4)..3*(c%4)+3.
QKV weights column-sharded, Wo row-sharded (Megatron); host sums the 4
partial outputs per batch and adds bo.

v3: measured on HW that 64-contraction matmuls cost ~2x per streamed
column vs full 128-contraction ones (563ns/pair vs 143.6ns single), so
every matmul here is built as a full 128-contraction:
  - Q^T stored zero-padded per head (qt_z0=[qh0;0], qt_z1=[0;qh1],
    qt_z2=[qh2;0]); K^T packed (kt_a=[kh0;kh1], kt_b=[kh2;0]). Scores
    for head h are then single full matmuls sharing the kt weights.
  - out_b / wo2 (head-2 rows of the output projection) zero-padded to
    128 partitions.
Everything bf16 (halves DMA, no fp32r short-stream penalty), exp on ACT
(measured 582ns per [128,1024] tile -> 56us/rep floor), and the PE
stream is software-pipelined: AV lags exp by one group, projections and
the previous block's output projection fill the gaps.
"""

import sys

sys.path.insert(0, "/opt/trn_rl_repo")

from contextlib import ExitStack

import ml_dtypes
import numpy as np

import concourse.bacc as bacc
import concourse.bass as bass
import concourse.tile as tile
from concourse import mybir
from concourse.bass_utils import run_bass_kernel_spmd

F32 = mybir.dt.float32
BF16 = mybir.dt.bfloat16

S = 2048  # sequence length
D = 768  # model dim
HP = 3  # heads per core
DK = 64  # head dim
DO = HP * DK  # 192 out-cols per core
KT = D // 128  # 6 contraction tiles for projections
NB = S // 512  # 4 sequence blocks of 512
NKT = S // 128  # 16 kpos tiles
G = NKT // 2  # 8 groups of 2 kpos tiles
VW = HP * 128  # 384: per head [V(64) | ones(1) | zeros(63)] = 128-wide lhsT


TINY_ACT = False  # diagnostic: shrink exp activations to ~zero work
TINY_DVE = False  # diagnostic: shrink DVE ops to ~zero work


def emit_kernel(nc, tc, t, reps=1):
    ctx = ExitStack()
    sb = ctx.enter_context(tc.tile_pool(name="sb", bufs=1))
    xp = ctx.enter_context(tc.tile_pool(name="xp", bufs=1))
    pt_pool = ctx.enter_context(tc.tile_pool(name="ptp", bufs=2))
    work = ctx.enter_context(tc.tile_pool(name="work", bufs=2))
    ps = ctx.enter_context(tc.tile_pool(name="ps", bufs=1, space=bass.MemorySpace.PSUM))

    AW = 4 if TINY_ACT else 512
    DW = 4 if TINY_DVE else 512
    DV = 4 if TINY_DVE else 64

    def act_exp(dst, src):
        nc.scalar.activation(
            dst[:, :, 0:AW], src[:, :, 0:AW],
            mybir.ActivationFunctionType.Exp, scale=0.125,
        )

    # ---- persistent SBUF tensors ----
    wq_sb = sb.tile([128, KT, 256], BF16)  # cols 0:192 = Wq, 192:256 zero
    wk_sb = sb.tile([128, KT, 256], BF16)  # (uniform 128-wide lhsT slices)
    # rhs operands padded to 512 streamed cols: short streams measured ~3x
    # slower per column on HW (192-col matmul 424ns vs 512-col 143.6ns)
    wv_sb = sb.tile([128, KT, 512], BF16)  # wv in cols 0:192, zeros after
    wo1_sb = sb.tile([128, D], BF16)  # Wo rows 0:128 (lhsT for y^T outproj)
    wo2_sb = sb.tile([128, D], BF16)  # Wo rows 128:192 at 0:64, zeros below
    bq_sb = sb.tile([128, 2], F32)
    bk_sb = sb.tile([128, 2], F32)
    bv_bc = sb.tile([128, DO], F32)  # bv broadcast to 128 partitions
    qt_z0 = sb.tile([128, S], BF16)  # [Q^T h0; 0]
    qt_z1 = sb.tile([128, S], BF16)  # [0; Q^T h1]
    qt_z2 = sb.tile([128, S], BF16)  # [Q^T h2; 0]
    kt_a = sb.tile([128, S], BF16)  # [K^T h0; K^T h1]
    kt_b = sb.tile([128, S], BF16)  # [K^T h2; 0]
    v_sb = sb.tile([128, NKT, VW], BF16)  # k-major V with ones cols
    out_a = sb.tile([128, S], BF16)  # attention out^T rows 0:128 (h0,h1)
    out_b = sb.tile([128, S], BF16)  # rows 128:192 (h2) at 0:64, zeros below

    # ---- weight / bias loads + one-time zero/ones fills ----
    nc.sync.dma_start(
        wk_sb[:, :, 0:DO], t["wk"].ap().rearrange("(k p) o -> p k o", p=128)
    )
    nc.sync.dma_start(
        wq_sb[:, :, 0:DO], t["wq"].ap().rearrange("(k p) o -> p k o", p=128)
    )
    nc.vector.memset(wk_sb[:, :, DO:256], 0.0)
    nc.vector.memset(wq_sb[:, :, DO:256], 0.0)
    nc.sync.dma_start(
        wv_sb[:, :, 0:DO], t["wv"].ap().rearrange("(k p) o -> p k o", p=128)
    )
    nc.sync.dma_start(wo1_sb[:], t["wo"].ap()[0:128, :])
    nc.sync.dma_start(wo2_sb[0:64, :], t["wo"].ap()[128:DO, :])
    nc.sync.dma_start(bq_sb[:, 0:1], t["bq"].ap()[0:128, :])
    nc.sync.dma_start(bq_sb[0:64, 1:2], t["bq"].ap()[128:DO, :])
    nc.sync.dma_start(bk_sb[:, 0:1], t["bk"].ap()[0:128, :])
    nc.sync.dma_start(bk_sb[0:64, 1:2], t["bk"].ap()[128:DO, :])
    nc.sync.dma_start(bv_bc[:], t["bv"].ap().unsqueeze(1).to_broadcast([1, 128, DO]))
    nc.vector.memset(wv_sb[:, :, DO:512], 0.0)
    nc.vector.memset(wo2_sb[64:128, :], 0.0)
    nc.vector.memset(qt_z0[64:128, :], 0.0)
    nc.vector.memset(qt_z1[0:64, :], 0.0)
    nc.vector.memset(qt_z2[64:128, :], 0.0)
    nc.vector.memset(kt_b[64:128, :], 0.0)
    nc.vector.memset(out_b[64:128, :], 0.0)
    for h in range(HP):
        nc.vector.memset(v_sb[:, :, h * 128 + 64 : h * 128 + 65], 1.0)
        nc.vector.memset(v_sb[:, :, h * 128 + 65 : h * 128 + 128], 0.0)

    # ---- per-tensor x tiles (all three live concurrently) ----
    xk_t = [xp.tile([128, S], BF16, name=f"xk{k}", tag=f"xk{k}") for k in range(KT)]
    xq_t = [xp.tile([128, S], BF16, name=f"xq{k}", tag=f"xq{k}") for k in range(KT)]
    xv_t = [xp.tile([128, S], BF16, name=f"xv{k}", tag=f"xv{k}") for k in range(KT)]

    def load_chunk(tiles, dram, c0, w):
        for k in range(KT):
            nc.gpsimd.dma_start(
                tiles[k][:, c0 : c0 + w],
                dram.ap()[k * 128 : k * 128 + 128, c0 : c0 + w],
            )

    def proj_k_a(nb):
        """K heads 0,1 -> kt_a (packed)."""
        nb0 = nb * 512
        pq1 = ps.tile([128, 2, 512], F32, tag="A", bufs=2, name="pq1")
        for k in range(KT):
            nc.tensor.matmul(
                pq1[:, 0, :], wk_sb[:, k, 0:128], xk_t[k][:, nb0 : nb0 + 512],
                start=(k == 0), stop=(k == KT - 1),
            )
        nc.vector.tensor_scalar_add(
            kt_a[:, nb0 : nb0 + DW], pq1[:, 0, 0:DW], bk_sb[:, 0:1]
        )

    def proj_k_b(nb):
        """K head 2 -> kt_b rows 0:64."""
        nb0 = nb * 512
        pq2 = ps.tile([128, 512], F32, tag="B", bufs=1, name="pq2")
        for k in range(KT):
            nc.tensor.matmul(
                pq2[:], wk_sb[:, k, 128:256], xk_t[k][:, nb0 : nb0 + 512],
                start=(k == 0), stop=(k == KT - 1),
            )
        nc.vector.tensor_scalar_add(
            kt_b[0:64, nb0 : nb0 + DW], pq2[0:64, 0:DW], bk_sb[0:64, 1:2]
        )

    def proj_q_a(nb):
        """Q heads 0,1 -> zero-padded qt_z0 / qt_z1."""
        nb0 = nb * 512
        pq1 = ps.tile([128, 2, 512], F32, tag="A", bufs=2, name="pq1")
        for k in range(KT):
            nc.tensor.matmul(
                pq1[:, 0, :], wq_sb[:, k, 0:128], xq_t[k][:, nb0 : nb0 + 512],
                start=(k == 0), stop=(k == KT - 1),
            )
        nc.vector.tensor_scalar_add(
            qt_z0[0:64, nb0 : nb0 + DW], pq1[0:64, 0, 0:DW], bq_sb[0:64, 0:1]
        )
        nc.vector.tensor_scalar_add(
            qt_z1[64:128, nb0 : nb0 + DW], pq1[64:128, 0, 0:DW], bq_sb[64:128, 0:1]
        )

    def proj_q_b(nb):
        """Q head 2 -> qt_z2 rows 0:64."""
        nb0 = nb * 512
        pq2 = ps.tile([128, 512], F32, tag="B", bufs=1, name="pq2")
        for k in range(KT):
            nc.tensor.matmul(
                pq2[:], wq_sb[:, k, 128:256], xq_t[k][:, nb0 : nb0 + 512],
                start=(k == 0), stop=(k == KT - 1),
            )
        nc.vector.tensor_scalar_add(
            qt_z2[0:64, nb0 : nb0 + DW], pq2[0:64, 0:DW], bq_sb[0:64, 1:2]
        )

    def proj_v_st(st):
        pv = ps.tile([128, 512], F32, tag="B", bufs=1, name="pv")
        for k in range(KT):
            nc.tensor.matmul(
                pv[:], xv_t[k][:, st * 128 : st * 128 + 128], wv_sb[:, k, :],
                start=(k == 0), stop=(k == KT - 1),
            )
        dst = v_sb[:, st, :].rearrange("p (h c) -> p h c", h=HP)[:, :, 0:DV]
        nc.vector.tensor_add(
            dst,
            pv[:, 0:DO].rearrange("p (h c) -> p h c", h=HP)[:, :, 0:DV],
            bv_bc[:].rearrange("p (h c) -> p h c", h=HP)[:, :, 0:DV],
        )

    def vslice(kt_i, h):
        return v_sb[:, kt_i, h * 128 : h * 128 + 128]

    def normalize(po, h, q0):
        """row 64 of po = denominator; write normalized out^T rows."""
        dtmp = work.tile([65, 512], F32, name="dtmp", tag="dtmp")
        nc.vector.reciprocal(dtmp[64:65, 0:DW], po[64:65, 0:DW])
        dbc = work.tile([64, 512], F32, name="dbc", tag="dbc")
        nc.sync.dma_start(dbc[:], dtmp[64:65, :].unsqueeze(1).to_broadcast([1, 64, 512]))
        if h == 0:
            nc.vector.tensor_mul(out_a[0:64, q0 : q0 + DW], po[0:64, 0:DW], dbc[:, 0:DW])
        elif h == 2:
            nc.vector.tensor_mul(out_b[0:64, q0 : q0 + DW], po[0:64, 0:DW], dbc[:, 0:DW])
        else:
            nsb = work.tile([64, 512], BF16, name="nsb", tag="nsb")
            nc.vector.tensor_mul(nsb[:, 0:DW], po[0:64, 0:DW], dbc[:, 0:DW])
            nc.sync.dma_start(out_a[64:128, q0 : q0 + 512], nsb[:])

    def outproj_c(qb, c):
        """y^T[c*128:(c+1)*128, qb block]: wo as weights, out as rhs."""
        q0 = qb * 512
        c0 = c * 128
        py = ps.tile([128, 512], F32, tag="C", bufs=1, name="py")
        nc.tensor.matmul(
            py[:], wo1_sb[:, c0 : c0 + 128], out_a[:, q0 : q0 + 512],
            start=True, stop=False,
        )
        nc.tensor.matmul(
            py[:], wo2_sb[:, c0 : c0 + 128], out_b[:, q0 : q0 + 512],
            start=False, stop=True,
        )
        ysb = work.tile([128, 512], BF16, name="ysb", tag="ysb")
        nc.vector.tensor_copy(ysb[:, 0:DW], py[:, 0:DW])
        nc.sync.dma_start(t["y"].ap()[c0 : c0 + 128, q0 : q0 + 512], ysb[:])

    def emit_av(g, pt0, pt1, po0, po1):
        for kl in range(2):
            kt_i = g * 2 + kl
            nc.tensor.matmul(
                po0[:], vslice(kt_i, 0), pt0[:, kl, :],
                start=(kt_i == 0), stop=(kt_i == NKT - 1),
                skip_group_check=True,
            )
            nc.tensor.matmul(
                po1[:], vslice(kt_i, 1), pt1[:, kl, :],
                start=(kt_i == 0), stop=(kt_i == NKT - 1),
                skip_group_check=True,
            )

    def emit_av2(h, pt2, po2):
        for kl in range(2):
            kt_i = h * 2 + kl
            nc.tensor.matmul(
                po2[:], vslice(kt_i, 2), pt2[:, kl, :],
                start=(kt_i == 0), stop=(kt_i == NKT - 1),
                skip_group_check=True,
            )

    def attention_qb(qb, fillers_g, fillers_h):
        q0 = qb * 512
        po0 = ps.tile([128, 512], F32, tag="B2", bufs=2, name="po0")
        po1 = ps.tile([128, 512], F32, tag="B2", bufs=2, name="po1")
        pts = [None] * G
        for g in range(G):
            pss0 = ps.tile([128, 2, 512], F32, tag="A", bufs=2, name="pss0")
            pss1 = ps.tile([128, 2, 512], F32, tag="A", bufs=2, name="pss1")
            for kl in range(2):
                kk = (g * 2 + kl) * 128
                nc.tensor.matmul(
                    pss0[:, kl, :], kt_a[:, kk : kk + 128],
                    qt_z0[:, q0 : q0 + 512], start=True, stop=True,
                )
                nc.tensor.matmul(
                    pss1[:, kl, :], kt_a[:, kk : kk + 128],
                    qt_z1[:, q0 : q0 + 512], start=True, stop=True,
                )
            pt0 = pt_pool.tile([128, 2, 512], BF16, name="pt0", tag="pt0", bufs=3)
            act_exp(pt0, pss0)
            pt1 = pt_pool.tile([128, 2, 512], BF16, name="pt1", tag="pt1", bufs=3)
            act_exp(pt1, pss1)
            pts[g] = (pt0, pt1)
            for f in fillers_g.get(g, ()):
                f()
            if g > 1:
                emit_av(g - 2, *pts[g - 2], po0, po1)
        emit_av(G - 2, *pts[G - 2], po0, po1)
        emit_av(G - 1, *pts[G - 1], po0, po1)
        normalize(po0, 0, q0)
        normalize(po1, 1, q0)
        # head 2
        po2 = ps.tile([128, 512], F32, tag="B2", bufs=2, name="po2")
        pt2s = [None] * G
        for h in range(G):
            pss2 = ps.tile([128, 2, 512], F32, tag="A", bufs=2, name="pss2")
            for kl in range(2):
                kk = (h * 2 + kl) * 128
                nc.tensor.matmul(
                    pss2[:, kl, :], kt_b[:, kk : kk + 128],
                    qt_z2[:, q0 : q0 + 512], start=True, stop=True,
                )
            pt2 = pt_pool.tile([128, 2, 512], BF16, name="pt2", tag="pt2", bufs=3)
            act_exp(pt2, pss2)
            pt2s[h] = pt2
            for f in fillers_h.get(h, ()):
                f()
            if h > 1:
                emit_av2(h - 2, pt2s[h - 2], po2)
        emit_av2(G - 2, pt2s[G - 2], po2)
        emit_av2(G - 1, pt2s[G - 1], po2)
        normalize(po2, 2, q0)

    def Ka(nb):
        return lambda: proj_k_a(nb)

    def Kb(nb):
        return lambda: proj_k_b(nb)

    def Qa(nb):
        return lambda: proj_q_a(nb)

    def Qb(nb):
        return lambda: proj_q_b(nb)

    def V(st):
        return lambda: proj_v_st(st)

    def O(qb, c):
        return lambda: outproj_c(qb, c)

    for _ in range(reps):
        # DMA issue order: critical-path chunks first
        load_chunk(xk_t, t["xk"], 0, 1024)
        load_chunk(xq_t, t["xq"], 0, 1024)
        load_chunk(xv_t, t["xv"], 0, 512)
        load_chunk(xv_t, t["xv"], 512, 512)
        load_chunk(xk_t, t["xk"], 1024, 1024)
        load_chunk(xq_t, t["xq"], 1024, 1024)
        load_chunk(xv_t, t["xv"], 1024, 1024)

        # warmup: just enough K/Q for the first score groups
        proj_k_a(0)
        proj_q_a(0)

        attention_qb(
            0,
            fillers_g={
                1: [Ka(1), V(0), V(1)],
                2: [V(2), V(3)],
                3: [Ka(2), V(4), V(5)],
                4: [V(6), V(7)],
                5: [Ka(3), V(8), V(9)],
                6: [Kb(0), Kb(1), V(10), V(11)],
                7: [Kb(2), Kb(3), V(12), V(13), V(14), V(15), Qb(0)],
            },
            fillers_h={0: [Qa(1)], 1: [Qb(1)]},
        )
        attention_qb(
            1,
            fillers_g={0: [Qa(2)], 1: [Qb(2)],
                       2: [O(0, 0), O(0, 1), O(0, 2)],
                       3: [O(0, 3), O(0, 4), O(0, 5)]},
            fillers_h={},
        )
        attention_qb(
            2,
            fillers_g={0: [Qa(3)], 1: [Qb(3)],
                       2: [O(1, 0), O(1, 1), O(1, 2)],
                       3: [O(1, 3), O(1, 4), O(1, 5)]},
            fillers_h={},
        )
        attention_qb(
            3,
            fillers_g={2: [O(2, 0), O(2, 1), O(2, 2)],
                       3: [O(2, 3), O(2, 4), O(2, 5)]},
            fillers_h={},
        )
        for c in range(6):
            outproj_c(3, c)

    ctx.close()


_NC_CACHE = {}


def build_nc(reps=1):
    if reps in _NC_CACHE:
        return _NC_CACHE[reps]
    nc = bacc.Bacc("TRN2", target_bir_lowering=False, debug=False, num_devices=8)
    t = {}
    for name in ("xq", "xk", "xv"):
        t[name] = nc.dram_tensor(name, [D, S], BF16, kind="ExternalInput")
    for name in ("wq", "wk", "wv"):
        t[name] = nc.dram_tensor(name, [D, DO], BF16, kind="ExternalInput")
    t["wo"] = nc.dram_tensor("wo", [DO, D], BF16, kind="ExternalInput")
    for name in ("bq", "bk"):
        t[name] = nc.dram_tensor(name, [DO, 1], F32, kind="ExternalInput")
    t["bv"] = nc.dram_tensor("bv", [1, DO], F32, kind="ExternalInput")
    t["y"] = nc.dram_tensor("y", [D, S], BF16, kind="ExternalOutput")

    with tile.TileContext(nc) as tc:
        emit_kernel(nc, tc, t, reps=reps)
    nc.compile()
    _NC_CACHE[reps] = nc
    return nc


def make_in_maps(q, k, v, Wq, bq, Wk, bk, Wv, bv, Wo, bo):
    bf = ml_dtypes.bfloat16
    in_maps = []
    for c in range(8):
        b = c // 4
        hs = (c % 4) * DO
        in_maps.append(
            {
                "xq": np.ascontiguousarray(q[b].T).astype(bf),
                "xk": np.ascontiguousarray(k[b].T).astype(bf),
                "xv": np.ascontiguousarray(v[b].T).astype(bf),
                "wq": np.ascontiguousarray(Wq[:, hs : hs + DO]).astype(bf),
                "wk": np.ascontiguousarray(Wk[:, hs : hs + DO]).astype(bf),
                "wv": np.ascontiguousarray(Wv[:, hs : hs + DO]).astype(bf),
                "wo": np.ascontiguousarray(Wo[hs : hs + DO, :]).astype(bf),
                "bq": np.ascontiguousarray(bq[hs : hs + DO, None]).astype(np.float32),
                "bk": np.ascontiguousarray(bk[hs : hs + DO, None]).astype(np.float32),
                "bv": np.ascontiguousarray(bv[None, hs : hs + DO]).astype(np.float32),
            }
        )
    return in_maps


def kernel(q, k, v, Wq, bq, Wk, bk, Wv, bv, Wo, bo, _reps=1):
    q = np.asarray(q, dtype=np.float32)
    k = np.asarray(k, dtype=np.float32)
    v = np.asarray(v, dtype=np.float32)
    nc = build_nc(reps=_reps)
    in_maps = make_in_maps(q, k, v, np.asarray(Wq), np.asarray(bq), np.asarray(Wk),
                           np.asarray(bk), np.asarray(Wv), np.asarray(bv),
                           np.asarray(Wo), np.asarray(bo))
    res = run_bass_kernel_spmd(nc, in_maps, list(range(8)))
    B = q.shape[0]
    y = np.zeros((B, S, D), dtype=np.float32)
    for c in range(8):
        y[c // 4] += np.asarray(res.results[c]["y"], dtype=np.float32).T
    y += np.asarray(bo, dtype=np.float32)[None, None, :]
    return y


# revision 22
# speedup vs baseline: 1.5351x; 1.1725x over previous
"""Multi-head attention (B=2, S=2048, D=768, H=12) on 8 Trainium2 cores.

Sharding: core c handles batch b=c//4 and heads 3*(c%4)..3*(c%4)+3.
QKV weights column-sharded, Wo row-sharded (Megatron); host sums the 4
partial outputs per batch and adds bo.

v3: measured on HW that 64-contraction matmuls cost ~2x per streamed
column vs full 128-contraction ones (563ns/pair vs 143.6ns single), so
every matmul here is built as a full 128-contraction:
  - Q^T stored zero-padded per head (qt_z0=[qh0;0], qt_z1=[0;qh1],
    qt_z2=[qh2;0]); K^T packed (kt_a=[kh0;kh1], kt_b=[kh2;0]). Scores
    for head h are then single full matmuls sharing the kt weights.
  - out_b / wo2 (head-2 rows of the output projection) zero-padded to
    128 partitions.
Everything bf16 (halves DMA, no fp32r short-stream penalty), exp on ACT
(measured 582ns per [128,1024] tile -> 56us/rep floor), and the PE
stream is software-pipelined: AV lags exp by one group, projections and
the previous block's output projection fill the gaps.
"""

import sys

sys.path.insert(0, "/opt/trn_rl_repo")

from contextlib import ExitStack

import ml_dtypes
import numpy as np

import concourse.bacc as bacc
import concourse.bass as bass
import concourse.tile as tile
from concourse import mybir
from concourse.bass_utils import run_bass_kernel_spmd

F32 = mybir.dt.float32
BF16 = mybir.dt.bfloat16

S = 2048  # sequence length
D = 768  # model dim
HP = 3  # heads per core
DK = 64  # head dim
DO = HP * DK  # 192 out-cols per core
KT = D // 128  # 6 contraction tiles for projections
NB = S // 512  # 4 sequence blocks of 512
NKT = S // 128  # 16 kpos tiles
G = NKT // 2  # 8 groups of 2 kpos tiles
VW = HP * 128  # 384: per head [V(64) | ones(1) | zeros(63)] = 128-wide lhsT


TINY_ACT = False  # diagnostic: shrink exp activations to ~zero work
TINY_DVE = False  # diagnostic: shrink DVE ops to ~zero work


def emit_kernel(nc, tc, t, reps=1):
    ctx = ExitStack()
    sb = ctx.enter_context(tc.tile_pool(name="sb", bufs=1))
    xp = ctx.enter_context(tc.tile_pool(name="xp", bufs=1))
    pt_pool = ctx.enter_context(tc.tile_pool(name="ptp", bufs=2))
    work = ctx.enter_context(tc.tile_pool(name="work", bufs=2))
    ps = ctx.enter_context(tc.tile_pool(name="ps", bufs=1, space=bass.MemorySpace.PSUM))

    AW = 4 if TINY_ACT else 512
    DW = 4 if TINY_DVE else 512
    DV = 4 if TINY_DVE else 64

    def act_exp(dst, src):
        nc.scalar.activation(
            dst[:, :, 0:AW], src[:, :, 0:AW],
            mybir.ActivationFunctionType.Exp, scale=0.125,
        )

    # ---- persistent SBUF tensors ----
    wq_sb = sb.tile([128, KT, 256], BF16)  # cols 0:192 = Wq, 192:256 zero
    wk_sb = sb.tile([128, KT, 256], BF16)  # (uniform 128-wide lhsT slices)
    # rhs operands padded to 512 streamed cols: short streams measured ~3x
    # slower per column on HW (192-col matmul 424ns vs 512-col 143.6ns)
    wv_sb = sb.tile([128, KT, 512], BF16)  # wv in cols 0:192, zeros after
    wo1_sb = sb.tile([128, D], BF16)  # Wo rows 0:128 (lhsT for y^T outproj)
    wo2_sb = sb.tile([128, D], BF16)  # Wo rows 128:192 at 0:64, zeros below
    bq_sb = sb.tile([128, 2], F32)
    bk_sb = sb.tile([128, 2], F32)
    bv_bc = sb.tile([128, DO], F32)  # bv broadcast to 128 partitions
    qt_z0 = sb.tile([128, S], BF16)  # [Q^T h0; 0]
    qt_z1 = sb.tile([128, S], BF16)  # [0; Q^T h1]
    qt_z2 = sb.tile([128, S], BF16)  # [Q^T h2; 0]
    kt_a = sb.tile([128, S], BF16)  # [K^T h0; K^T h1]
    kt_b = sb.tile([128, S], BF16)  # [K^T h2; 0]
    v_sb = sb.tile([128, NKT, VW], BF16)  # k-major V with ones cols
    pt_sw = [
        sb.tile([128, G, 2, 512], BF16, name=f"pt_sw{i}", tag=f"pt_sw{i}")
        for i in range(2)
    ]  # P sweep double buffers
    out_a = sb.tile([128, S], BF16)  # attention out^T rows 0:128 (h0,h1)
    out_b = sb.tile([128, S], BF16)  # rows 128:192 (h2) at 0:64, zeros below

    # ---- weight / bias loads + one-time zero/ones fills ----
    nc.sync.dma_start(
        wk_sb[:, :, 0:DO], t["wk"].ap().rearrange("(k p) o -> p k o", p=128)
    )
    nc.sync.dma_start(
        wq_sb[:, :, 0:DO], t["wq"].ap().rearrange("(k p) o -> p k o", p=128)
    )
    nc.vector.memset(wk_sb[:, :, DO:256], 0.0)
    nc.vector.memset(wq_sb[:, :, DO:256], 0.0)
    nc.sync.dma_start(
        wv_sb[:, :, 0:DO], t["wv"].ap().rearrange("(k p) o -> p k o", p=128)
    )
    nc.sync.dma_start(wo1_sb[:], t["wo"].ap()[0:128, :])
    nc.sync.dma_start(wo2_sb[0:64, :], t["wo"].ap()[128:DO, :])
    nc.sync.dma_start(bq_sb[:, 0:1], t["bq"].ap()[0:128, :])
    nc.sync.dma_start(bq_sb[0:64, 1:2], t["bq"].ap()[128:DO, :])
    nc.sync.dma_start(bk_sb[:, 0:1], t["bk"].ap()[0:128, :])
    nc.sync.dma_start(bk_sb[0:64, 1:2], t["bk"].ap()[128:DO, :])
    nc.sync.dma_start(bv_bc[:], t["bv"].ap().unsqueeze(1).to_broadcast([1, 128, DO]))
    nc.vector.memset(wv_sb[:, :, DO:512], 0.0)
    nc.vector.memset(wo2_sb[64:128, :], 0.0)
    nc.vector.memset(qt_z0[64:128, :], 0.0)
    nc.vector.memset(qt_z1[0:64, :], 0.0)
    nc.vector.memset(qt_z2[64:128, :], 0.0)
    nc.vector.memset(kt_b[64:128, :], 0.0)
    nc.vector.memset(out_b[64:128, :], 0.0)
    for h in range(HP):
        nc.vector.memset(v_sb[:, :, h * 128 + 64 : h * 128 + 65], 1.0)
        nc.vector.memset(v_sb[:, :, h * 128 + 65 : h * 128 + 128], 0.0)

    # ---- per-tensor x tiles (all three live concurrently) ----
    xk_t = [xp.tile([128, S], BF16, name=f"xk{k}", tag=f"xk{k}") for k in range(KT)]
    xq_t = [xp.tile([128, S], BF16, name=f"xq{k}", tag=f"xq{k}") for k in range(KT)]
    xv_t = [xp.tile([128, S], BF16, name=f"xv{k}", tag=f"xv{k}") for k in range(KT)]

    def load_chunk(tiles, dram, c0, w):
        for k in range(KT):
            nc.gpsimd.dma_start(
                tiles[k][:, c0 : c0 + w],
                dram.ap()[k * 128 : k * 128 + 128, c0 : c0 + w],
            )

    def proj_k_a(nb):
        """K heads 0,1 -> kt_a (packed)."""
        nb0 = nb * 512
        pq1 = ps.tile([128, 2, 512], F32, tag="A", bufs=2, name="pq1")
        for k in range(KT):
            nc.tensor.matmul(
                pq1[:, 0, :], wk_sb[:, k, 0:128], xk_t[k][:, nb0 : nb0 + 512],
                start=(k == 0), stop=(k == KT - 1),
            )
        nc.vector.tensor_scalar_add(
            kt_a[:, nb0 : nb0 + DW], pq1[:, 0, 0:DW], bk_sb[:, 0:1]
        )

    def proj_k_b(nb):
        """K head 2 -> kt_b rows 0:64."""
        nb0 = nb * 512
        pq2 = ps.tile([128, 512], F32, tag="B", bufs=1, name="pq2")
        for k in range(KT):
            nc.tensor.matmul(
                pq2[:], wk_sb[:, k, 128:256], xk_t[k][:, nb0 : nb0 + 512],
                start=(k == 0), stop=(k == KT - 1),
            )
        nc.vector.tensor_scalar_add(
            kt_b[0:64, nb0 : nb0 + DW], pq2[0:64, 0:DW], bk_sb[0:64, 1:2]
        )

    def proj_q_a(nb):
        """Q heads 0,1 -> zero-padded qt_z0 / qt_z1."""
        nb0 = nb * 512
        pq1 = ps.tile([128, 2, 512], F32, tag="A", bufs=2, name="pq1")
        for k in range(KT):
            nc.tensor.matmul(
                pq1[:, 0, :], wq_sb[:, k, 0:128], xq_t[k][:, nb0 : nb0 + 512],
                start=(k == 0), stop=(k == KT - 1),
            )
        nc.vector.tensor_scalar_add(
            qt_z0[0:64, nb0 : nb0 + DW], pq1[0:64, 0, 0:DW], bq_sb[0:64, 0:1]
        )
        nc.vector.tensor_scalar_add(
            qt_z1[64:128, nb0 : nb0 + DW], pq1[64:128, 0, 0:DW], bq_sb[64:128, 0:1]
        )

    def proj_q_b(nb):
        """Q head 2 -> qt_z2 rows 0:64."""
        nb0 = nb * 512
        pq2 = ps.tile([128, 512], F32, tag="B", bufs=1, name="pq2")
        for k in range(KT):
            nc.tensor.matmul(
                pq2[:], wq_sb[:, k, 128:256], xq_t[k][:, nb0 : nb0 + 512],
                start=(k == 0), stop=(k == KT - 1),
            )
        nc.vector.tensor_scalar_add(
            qt_z2[0:64, nb0 : nb0 + DW], pq2[0:64, 0:DW], bq_sb[0:64, 1:2]
        )

    def proj_v_st(st):
        pv = ps.tile([128, 512], F32, tag="B", bufs=1, name="pv")
        for k in range(KT):
            nc.tensor.matmul(
                pv[:], xv_t[k][:, st * 128 : st * 128 + 128], wv_sb[:, k, :],
                start=(k == 0), stop=(k == KT - 1),
            )
        dst = v_sb[:, st, :].rearrange("p (h c) -> p h c", h=HP)[:, :, 0:DV]
        nc.vector.tensor_add(
            dst,
            pv[:, 0:DO].rearrange("p (h c) -> p h c", h=HP)[:, :, 0:DV],
            bv_bc[:].rearrange("p (h c) -> p h c", h=HP)[:, :, 0:DV],
        )

    def vslice(kt_i, h):
        return v_sb[:, kt_i, h * 128 : h * 128 + 128]

    def normalize(po, h, q0):
        """row 64 of po = denominator; write normalized out^T rows."""
        dtmp = work.tile([65, 512], F32, name="dtmp", tag="dtmp")
        nc.vector.reciprocal(dtmp[64:65, 0:DW], po[64:65, 0:DW])
        dbc = work.tile([64, 512], F32, name="dbc", tag="dbc")
        nc.sync.dma_start(dbc[:], dtmp[64:65, :].unsqueeze(1).to_broadcast([1, 64, 512]))
        if h == 0:
            nc.vector.tensor_mul(out_a[0:64, q0 : q0 + DW], po[0:64, 0:DW], dbc[:, 0:DW])
        elif h == 2:
            nc.vector.tensor_mul(out_b[0:64, q0 : q0 + DW], po[0:64, 0:DW], dbc[:, 0:DW])
        else:
            nsb = work.tile([64, 512], BF16, name="nsb", tag="nsb")
            nc.vector.tensor_mul(nsb[:, 0:DW], po[0:64, 0:DW], dbc[:, 0:DW])
            nc.sync.dma_start(out_a[64:128, q0 : q0 + 512], nsb[:])

    def outproj_c(qb, c):
        """y^T[c*128:(c+1)*128, qb block]: wo as weights, out as rhs."""
        q0 = qb * 512
        c0 = c * 128
        py = ps.tile([128, 512], F32, tag="C", bufs=1, name="py")
        nc.tensor.matmul(
            py[:], wo1_sb[:, c0 : c0 + 128], out_a[:, q0 : q0 + 512],
            start=True, stop=False,
        )
        nc.tensor.matmul(
            py[:], wo2_sb[:, c0 : c0 + 128], out_b[:, q0 : q0 + 512],
            start=False, stop=True,
        )
        ysb = work.tile([128, 512], BF16, name="ysb", tag="ysb")
        nc.vector.tensor_copy(ysb[:, 0:DW], py[:, 0:DW])
        nc.sync.dma_start(t["y"].ap()[c0 : c0 + 128, q0 : q0 + 512], ysb[:])

    def emit_av(g, pt0, pt1, po0, po1):
        for kl in range(2):
            kt_i = g * 2 + kl
            nc.tensor.matmul(
                po0[:], vslice(kt_i, 0), pt0[:, kl, :],
                start=(kt_i == 0), stop=(kt_i == NKT - 1),
                skip_group_check=True,
            )
            nc.tensor.matmul(
                po1[:], vslice(kt_i, 1), pt1[:, kl, :],
                start=(kt_i == 0), stop=(kt_i == NKT - 1),
                skip_group_check=True,
            )

    def emit_av2(h, pt2, po2):
        for kl in range(2):
            kt_i = h * 2 + kl
            nc.tensor.matmul(
                po2[:], vslice(kt_i, 2), pt2[:, kl, :],
                start=(kt_i == 0), stop=(kt_i == NKT - 1),
                skip_group_check=True,
            )

    sweep_ctr = [0]

    def attention_head(qb, h, kt_src, qt_src, fillers):
        """One head: 16 scores + 8 exps into a sweep buffer, then 16
        consecutive AV accumulates (chain-resume with intervening matmuls
        measured ~2.5x slower than uninterrupted chains)."""
        q0 = qb * 512
        sw = pt_sw[sweep_ctr[0] % 2]
        sweep_ctr[0] += 1
        for g in range(G):
            pss = ps.tile([128, 2, 512], F32, tag="A", bufs=2, name="pss")
            for kl in range(2):
                kk = (g * 2 + kl) * 128
                nc.tensor.matmul(
                    pss[:, kl, :], kt_src[:, kk : kk + 128],
                    qt_src[:, q0 : q0 + 512], start=True, stop=True,
                )
            act_exp(sw[:, g], pss)
            for f in fillers.get(g, ()):
                f()
        po = ps.tile([128, 512], F32, tag="B2", bufs=2, name="po")
        for kt_i in range(NKT):
            nc.tensor.matmul(
                po[:], vslice(kt_i, h), sw[:, kt_i // 2, kt_i % 2, :],
                start=(kt_i == 0), stop=(kt_i == NKT - 1),
                skip_group_check=True,
            )
        normalize(po, h, q0)

    def attention_qb(qb, fillers_g, fillers_h):
        f0 = {g: fillers_g.get(g, ()) for g in range(G)}
        f1 = {g: fillers_g.get(g + G, ()) for g in range(G)}
        attention_head(qb, 0, kt_a, qt_z0, f0)
        attention_head(qb, 1, kt_a, qt_z1, f1)
        attention_head(qb, 2, kt_b, qt_z2, fillers_h)

    def Ka(nb):
        return lambda: proj_k_a(nb)

    def Kb(nb):
        return lambda: proj_k_b(nb)

    def Qa(nb):
        return lambda: proj_q_a(nb)

    def Qb(nb):
        return lambda: proj_q_b(nb)

    def V(st):
        return lambda: proj_v_st(st)

    def O(qb, c):
        return lambda: outproj_c(qb, c)

    for _ in range(reps):
        # DMA issue order: critical-path chunks first
        load_chunk(xk_t, t["xk"], 0, 1024)
        load_chunk(xq_t, t["xq"], 0, 1024)
        load_chunk(xv_t, t["xv"], 0, 512)
        load_chunk(xv_t, t["xv"], 512, 512)
        load_chunk(xk_t, t["xk"], 1024, 1024)
        load_chunk(xq_t, t["xq"], 1024, 1024)
        load_chunk(xv_t, t["xv"], 1024, 1024)

        # warmup: just enough K/Q for the first score groups
        proj_k_a(0)
        proj_q_a(0)

        attention_qb(
            0,
            fillers_g={
                0: [Ka(1)], 1: [V(0), V(1)], 2: [Ka(2)], 3: [V(2), V(3)],
                4: [Ka(3)], 5: [V(4), V(5)], 6: [V(6), V(7)],
                7: [V(8), V(9), V(10), V(11), V(12), V(13), V(14), V(15)],
                8: [Kb(0)], 9: [Kb(1)], 10: [Kb(2)], 11: [Kb(3)],
                12: [Qb(0)], 14: [Qa(1)],
            },
            fillers_h={0: [Qb(1)]},
        )
        attention_qb(
            1,
            fillers_g={0: [Qa(2)], 1: [Qb(2)],
                       2: [O(0, 0)], 3: [O(0, 1)], 4: [O(0, 2)],
                       5: [O(0, 3)], 6: [O(0, 4)], 7: [O(0, 5)]},
            fillers_h={},
        )
        attention_qb(
            2,
            fillers_g={0: [Qa(3)], 1: [Qb(3)],
                       2: [O(1, 0)], 3: [O(1, 1)], 4: [O(1, 2)],
                       5: [O(1, 3)], 6: [O(1, 4)], 7: [O(1, 5)]},
            fillers_h={},
        )
        attention_qb(
            3,
            fillers_g={2: [O(2, 0)], 3: [O(2, 1)], 4: [O(2, 2)],
                       5: [O(2, 3)], 6: [O(2, 4)], 7: [O(2, 5)]},
            fillers_h={},
        )
        for c in range(6):
            outproj_c(3, c)

    ctx.close()


_NC_CACHE = {}


def build_nc(reps=1):
    if reps in _NC_CACHE:
        return _NC_CACHE[reps]
    nc = bacc.Bacc("TRN2", target_bir_lowering=False, debug=False, num_devices=8)
    t = {}
    for name in ("xq", "xk", "xv"):
        t[name] = nc.dram_tensor(name, [D, S], BF16, kind="ExternalInput")
    for name in ("wq", "wk", "wv"):
        t[name] = nc.dram_tensor(name, [D, DO], BF16, kind="ExternalInput")
    t["wo"] = nc.dram_tensor("wo", [DO, D], BF16, kind="ExternalInput")
    for name in ("bq", "bk"):
        t[name] = nc.dram_tensor(name, [DO, 1], F32, kind="ExternalInput")
    t["bv"] = nc.dram_tensor("bv", [1, DO], F32, kind="ExternalInput")
    t["y"] = nc.dram_tensor("y", [D, S], BF16, kind="ExternalOutput")

    with tile.TileContext(nc) as tc:
        emit_kernel(nc, tc, t, reps=reps)
    nc.compile()
    _NC_CACHE[reps] = nc
    return nc


def make_in_maps(q, k, v, Wq, bq, Wk, bk, Wv, bv, Wo, bo):
    bf = ml_dtypes.bfloat16
    in_maps = []
    for c in range(8):
        b = c // 4
        hs = (c % 4) * DO
        in_maps.append(
            {
                "xq": np.ascontiguousarray(q[b].T).astype(bf),
                "xk": np.ascontiguousarray(k[b].T).astype(bf),
                "xv": np.ascontiguousarray(v[b].T).astype(bf),
                "wq": np.ascontiguousarray(Wq[:, hs : hs + DO]).astype(bf),
                "wk": np.ascontiguousarray(Wk[:, hs : hs + DO]).astype(bf),
                "wv": np.ascontiguousarray(Wv[:, hs : hs + DO]).astype(bf),
                "wo": np.ascontiguousarray(Wo[hs : hs + DO, :]).astype(bf),
                "bq": np.ascontiguousarray(bq[hs : hs + DO, None]).astype(np.float32),
                "bk": np.ascontiguousarray(bk[hs : hs + DO, None]).astype(np.float32),
                "bv": np.ascontiguousarray(bv[None, hs : hs + DO]).astype(np.float32),
            }
        )
    return in_maps


def kernel(q, k, v, Wq, bq, Wk, bk, Wv, bv, Wo, bo, _reps=1):
    q = np.asarray(q, dtype=np.float32)
    k = np.asarray(k, dtype=np.float32)
    v = np.asarray(v, dtype=np.float32)
    nc = build_nc(reps=_reps)
    in_maps = make_in_maps(q, k, v, np.asarray(Wq), np.asarray(bq), np.asarray(Wk),
                           np.asarray(bk), np.asarray(Wv), np.asarray(bv),
                           np.asarray(Wo), np.asarray(bo))
    res = run_bass_kernel_spmd(nc, in_maps, list(range(8)))
    B = q.shape[0]
    y = np.zeros((B, S, D), dtype=np.float32)
    for c in range(8):
        y[c // 4] += np.asarray(res.results[c]["y"], dtype=np.float32).T
    y += np.asarray(bo, dtype=np.float32)[None, None, :]
    return y
